# revision 9
# baseline (speedup 1.0000x reference)
"""Trainium2 Bass kernel for the DescriptorLoss dual-softmax loss.

Math (per batch element b):
    des1 = p1[b][:, y1, x1]            # [C=256, N=3540]
    des2 = p2[b][:, y2, x2]            # [C, N]
    dist = TEMP * des1.T @ des2        # [N, N]
    loss_b = 2*trace(dist) - sum_m lse_row[m] - sum_n lse_col[n]
    loss   = -(sum_b loss_b) / (B*N)

Sharding: data-parallel over the batch dim, one batch element per
NeuronCore (B == 8 == n_cores).  Host gathers descriptors and quantizes
to fp8e4m3 in the DoubleRow interleaved layout, runs the SPMD program,
and combines the per-core partial sums.

Per-core engine assignment (per 128-row m-tile, N=3540 columns):
    PE  : dist tile in ONE fp8 DoubleRow matmul pass (contraction 256 as
          128 partitions x 2 interleaved rows, 2x throughput), 7 chunks
          of <=512 cols into a single [128,3540] PSUM tile; plus a
          512-col ones-matmul column-sum accumulated in a 1-bank PSUM
          strip (psC) across all m-tiles (deferred one tile so PE never
          waits on this tile's exp).
    ACT : exact exp of cols [0, A_W) in ONE instruction with accum_out
          row sums.
    DVE : Schraudolph bit-trick exp of cols [A_W, N) (tensor_scalar
          (x*S + B) -> int16 bitcast bf16, ~2% rel err, fine for the
          2e-2 loss tolerance, validated vs ref); in-place rowsum pass
          over those cols (tensor_scalar 4x mode with accum_out); and
          colacc += exp over cols [0, CVV) (bf16 tensor_tensor, 2x).
    Pool: (GPSIMD cannot touch PSUM, nor run stt) colacc += exp over
          [CVV, CV) via bf16 tensor_tensor adds.  The exact-diag
          scalar_tensor_tensor runs on DVE in 28 small chunks.

Finalize: rowsums -> Ln+accum; colacc column sums via ones-matmuls into
4 partition-rows (0/32/64/96) of a [128,757] PSUM tile pre-memset to
1.0 so one full-width Ln+accum covers them (Ln(1)=0 elsewhere); psC
strip Ln'd separately; everything partition-reduced by one f32
ones-matmul and DMA'd out as [1,11].
"""

import numpy as np
import ml_dtypes

B = 8
C = 256
N = 3540
TEMP = 0.2
MT = 128
NT = (N + MT - 1) // MT          # 28 m-tiles (last has 84 rows)
NP = 3552                         # fp8 pair-dim stride must be 16B-aligned

A_W = 2802                        # ACT exact-exp columns [0, A_W)
# DVE Schraudolph columns [A_W, N)
CVV = 1702                        # DVE colacc [0, CVV)
CV = 3028                         # Pool colacc [CVV, CV); PE colsum [CV, N)
assert N - CV == 512

# Schraudolph constants: i16 = trunc(raw_dot * SCH_S + SCH_B); bitcast bf16
# approximates exp(TEMP * raw_dot).  C = -6.5 calibrated for minimal bias.
SCH_S = TEMP * 128.0 / float(np.log(2.0))
SCH_B = 16256.0 - 6.5

MM_N = 512
DIAG_W = (2 * N + NT - 1) // NT   # 253-wide diag chunk per m-tile
# fin columns: 0 diag partials, 1 rowlog, 2 collog(psF2), 3 collog(psC)
FIN_W = 4

_prog_cache = {}


def _mm_chunks(width):
    out = []
    off = 0
    while off < width:
        w = min(MM_N, width - off)
        out.append((off, w))
        off += w
    return out


def _build_program():
    import contextlib
    import concourse.bacc as bacc
    import concourse.tile as tile
    from concourse import mybir

    dt = mybir.dt
    f32 = dt.float32
    bf16 = dt.bfloat16
    i16 = dt.int16
    fp8 = dt.float8e4
    Exp = mybir.ActivationFunctionType.Exp
    Ln = mybir.ActivationFunctionType.Ln
    MULT = mybir.AluOpType.mult
    ADD = mybir.AluOpType.add
    DR = mybir.MatmulPerfMode.DoubleRow

    nc = bacc.Bacc(
        "TRN2", target_bir_lowering=False, debug=False, num_devices=B)
    d1 = nc.dram_tensor("d1", [MT, 2, NP], fp8, kind="ExternalInput")
    d2 = nc.dram_tensor("d2", [MT, 2, NP], fp8, kind="ExternalInput")
    b1 = nc.dram_tensor("b1", [MT, 2 * N], bf16, kind="ExternalInput")
    b2 = nc.dram_tensor("b2", [MT, 2 * N], bf16, kind="ExternalInput")
    out = nc.dram_tensor("out", [1, FIN_W], f32, kind="ExternalOutput")

    with tile.TileContext(nc) as tc:
        with (
            tc.tile_pool(name="persist", bufs=1) as persist,
            tc.tile_pool(name="etiles", bufs=2) as etiles,
            tc.tile_pool(name="small", bufs=1) as small,
        ):
            d1_sb = persist.tile([MT, 2, NP], fp8, name="d1_sb")
            d2_sb = persist.tile([MT, 2, NP], fp8, name="d2_sb")
            b1_sb = persist.tile([MT, 2 * N], bf16, name="b1_sb")
            b2_sb = persist.tile([MT, 2 * N], bf16, name="b2_sb")

            # fp8 operands first (tile 0 needs all of d2 + head of d1).
            nc.sync.dma_start(out=d2_sb, in_=d2[:, :, :])
            nc.scalar.dma_start(out=d1_sb[:, :, 0:512], in_=d1[:, :, 0:512])
            nc.scalar.dma_start(out=d1_sb[:, :, 512:NP], in_=d1[:, :, 512:NP])
            # bf16 copies (diag only; first chunk needed at m-tile 0, but
            # only 253 cols worth per tile).
            nc.sync.dma_start(out=b1_sb, in_=b1[:, :])
            nc.scalar.dma_start(out=b2_sb, in_=b2[:, :])

            colacc = persist.tile([MT, CV], bf16, name="colacc")
            nc.vector.memset(colacc, 0.0)

            # 0.5-init: rows of the last (84-row) m-tile that do not exist
            # leave 0.5 + 0.5 = 1.0 -> Ln contributes 0.
            rs_act = persist.tile([MT, NT], f32, name="rs_act")
            nc.vector.memset(rs_act, 0.5)
            rs_dve = persist.tile([MT, NT], f32, name="rs_dve")
            nc.vector.memset(rs_dve, 0.5)

            fin = small.tile([MT, FIN_W], f32, name="fin")
            nc.vector.memset(fin, 0.0)
            diagparts = small.tile([MT, NT], f32, name="diagparts")
            nc.vector.memset(diagparts, 0.0)

            ones_bf = small.tile([MT, 1], bf16, name="ones_bf")
            nc.vector.memset(ones_bf, 1.0)
            ones_f32 = small.tile([MT, 1], f32, name="ones_f32")
            nc.vector.memset(ones_f32, 1.0)

            scratchD = small.tile([MT, DIAG_W], bf16, name="scratchD")

            with contextlib.ExitStack() as pscctx:
                pscp = pscctx.enter_context(
                    tc.tile_pool(name="psc", bufs=1, space="PSUM"))
                psC = pscp.tile([1, 512], f32, tag="psC", name="psC")

                with tc.tile_pool(name="ps", bufs=1, space="PSUM") as psp:
                    prev = None  # (e_tile, mp) pending psC colsum matmul
                    for t in range(NT):
                        m0 = t * MT
                        mp = min(MT, N - m0)
                        ps = psp.tile([MT, N], f32, tag="ps", name="ps")
                        for (off, w) in _mm_chunks(N):
                            nc.tensor.matmul(
                                ps[:mp, off:off + w],
                                lhsT=d1_sb[:, :, m0:m0 + mp],
                                rhs=d2_sb[:, :, off:off + w],
                                start=True, stop=True, perf_mode=DR)
                        # deferred psC colsum for the previous tile keeps PE
                        # from stalling on this tile's DVE work
                        if prev is not None:
                            pe, pmp = prev
                            nc.tensor.matmul(
                                psC[0:1, :], lhsT=ones_bf[:pmp],
                                rhs=pe[:pmp, CV:N],
                                start=(t == 1), stop=False)

                        e = etiles.tile([MT, N], bf16, tag="e", name="e")
                        # ACT: exact exp + rowsums for [0, A_W)
                        nc.scalar.activation(
                            out=e[:mp, 0:A_W], in_=ps[:mp, 0:A_W], func=Exp,
                            scale=TEMP, accum_out=rs_act[:mp, t:t + 1])
                        # DVE: Schraudolph exp for [A_W, N)
                        nc.vector.tensor_scalar(
                            out=e[:mp, A_W:N].bitcast(i16),
                            in0=ps[:mp, A_W:N],
                            scalar1=SCH_S, scalar2=SCH_B, op0=MULT, op1=ADD)
                        # DVE: in-place rowsum pass over the Schraudolph cols
                        nc.vector.tensor_scalar(
                            out=e[:mp, A_W:N], in0=e[:mp, A_W:N],
                            scalar1=1.0, scalar2=None, op0=MULT, op1=ADD,
                            accum_out=rs_dve[:mp, t:t + 1])
                        # DVE: colacc [0, CVV)
                        nc.vector.tensor_add(
                            colacc[:mp, 0:CVV], colacc[:mp, 0:CVV],
                            e[:mp, 0:CVV])
                        # Pool: colacc [CVV, CV)
                        nc.gpsimd.tensor_add(
                            colacc[:mp, CVV:CV], colacc[:mp, CVV:CV],
                            e[:mp, CVV:CV])
                        # DVE: one diag chunk per tile (stt unsupported on Pool)
                        s = t * DIAG_W
                        w = min(DIAG_W, 2 * N - s)
                        nc.vector.scalar_tensor_tensor(
                            out=scratchD[:, 0:w], in0=b1_sb[:, s:s + w],
                            scalar=1.0, in1=b2_sb[:, s:s + w],
                            op0=MULT, op1=MULT,
                            accum_out=diagparts[:, t:t + 1])
                        prev = (e, mp)
                    pe, pmp = prev
                    nc.tensor.matmul(
                        psC[0:1, :], lhsT=ones_bf[:pmp], rhs=pe[:pmp, CV:N],
                        start=False, stop=True)

                # ---- finalize ----
                # diag partials: fold the 28 chunk sums into fin[:,0]
                nc.vector.tensor_scalar(
                    out=diagparts, in0=diagparts, scalar1=1.0, scalar2=None,
                    op0=MULT, op1=ADD, accum_out=fin[:, 0:1])
                rowsums = small.tile([MT, NT], f32, name="rowsums")
                nc.vector.tensor_add(rowsums, rs_act, rs_dve)
                rl = small.tile([MT, NT], f32, name="rl")
                nc.scalar.activation(out=rl, in_=rowsums, func=Ln,
                                     accum_out=fin[:, 1:2])

                with tc.tile_pool(name="psF", bufs=1, space="PSUM") as psF:
                    # colacc column sums: 4 blocks of 757 cols, one per
                    # partition-row 0/32/64/96; unused partitions memset to
                    # 1.0 so a single full-width Ln contributes 0 there.
                    psF2 = psF.tile([MT, CV // 4], f32, tag="psF2",
                                    name="psF2")
                    nc.vector.memset(psF2, 1.0)
                    for j in range(4):
                        base = j * (CV // 4)
                        for (off, w) in _mm_chunks(CV // 4):
                            nc.tensor.matmul(
                                psF2[32 * j:32 * j + 1, off:off + w],
                                lhsT=ones_bf,
                                rhs=colacc[:, base + off:base + off + w],
                                start=True, stop=True,
                                tile_position=(0, 32 * j))
                    clF = small.tile([MT, CV // 4], f32, name="clF")
                    nc.scalar.activation(out=clF, in_=psF2, func=Ln,
                                         accum_out=fin[:, 2:3])
                    clC = small.tile([1, 512], f32, name="clC")
                    nc.scalar.activation(out=clC, in_=psC[0:1, :], func=Ln,
                                         accum_out=fin[0:1, 3:4])

                    dr_ps = psF.tile([1, FIN_W], f32, tag="drps", name="drps")
                    nc.tensor.matmul(dr_ps[0:1, :], lhsT=ones_f32,
                                     rhs=fin, start=True, stop=True)
                    outsb = small.tile([1, FIN_W], f32, name="outsb")
                    nc.vector.tensor_copy(outsb, dr_ps[0:1, :])
                    nc.sync.dma_start(out=out[:, :], in_=outsb)

    nc.compile()
    return nc


def _get_program():
    if "nc" not in _prog_cache:
        _prog_cache["nc"] = _build_program()
    return _prog_cache["nc"]


def _prep_in_maps(inputs):
    p1 = np.asarray(inputs["p1"], dtype=np.float32)
    p2 = np.asarray(inputs["p2"], dtype=np.float32)
    y1 = np.asarray(inputs["y1"]).astype(np.int64)
    x1 = np.asarray(inputs["x1"]).astype(np.int64)
    y2 = np.asarray(inputs["y2"]).astype(np.int64)
    x2 = np.asarray(inputs["x2"]).astype(np.int64)

    des1 = p1[:, :, y1, x1]                      # [B, C, N] f32
    des2 = p2[:, :, y2, x2]
    # DoubleRow layout: [128, 2, N], element (i*128+p, n) -> [p, i, n]
    dr1 = des1.reshape(B, 2, MT, N).transpose(0, 2, 1, 3)
    dr2 = des2.reshape(B, 2, MT, N).transpose(0, 2, 1, 3)
    pad = np.zeros((B, MT, 2, NP - N), np.float32)
    f8_1 = np.ascontiguousarray(np.concatenate([dr1, pad], axis=3)).astype(
        ml_dtypes.float8_e4m3fn)
    f8_2 = np.ascontiguousarray(np.concatenate([dr2, pad], axis=3)).astype(
        ml_dtypes.float8_e4m3fn)
    bf_1 = np.ascontiguousarray(dr1.reshape(B, MT, 2 * N)).astype(
        ml_dtypes.bfloat16)
    bf_2 = np.ascontiguousarray(dr2.reshape(B, MT, 2 * N)).astype(
        ml_dtypes.bfloat16)
    return [
        {"d1": f8_1[b], "d2": f8_2[b], "b1": bf_1[b], "b2": bf_2[b]}
        for b in range(B)
    ]


def _combine(results):
    total = 0.0
    for b in range(B):
        v = np.asarray(results[b]["out"], dtype=np.float64).ravel()
        total += 2.0 * TEMP * v[0] - v[1] - (v[2] + v[3])
    return np.float32(-total / (B * N))


def kernel(**inputs) -> np.ndarray:
    from concourse.bass_utils import run_bass_kernel_spmd

    nc = _get_program()
    in_maps = _prep_in_maps(inputs)
    res = run_bass_kernel_spmd(nc, in_maps, list(range(B)))
    return _combine(res.results)


# revision 11
# speedup vs baseline: 1.1545x; 1.1545x over previous
"""Trainium2 Bass kernel for the DescriptorLoss dual-softmax loss.

Math (per batch element b):
    des1 = p1[b][:, y1, x1]            # [C=256, N=3540]
    des2 = p2[b][:, y2, x2]            # [C, N]
    dist = TEMP * des1.T @ des2        # [N, N]
    loss_b = 2*trace(dist) - sum_m lse_row[m] - sum_n lse_col[n]
    loss   = -(sum_b loss_b) / (B*N)

Sharding: data-parallel over the batch dim, one batch element per
NeuronCore (B == 8 == n_cores).

Per-core pipeline, per 128-row m-tile over two PSUM regions
R0=[0,1536), R1=[1536,3540) (measured-rate balanced ~3.6us/tile):
    PE  : dist via fp8e4m3 DoubleRow matmuls (contraction 256 = 128
          partitions x 2 interleaved; block layouts keep the pair
          stride small: lhsT [128,28,2,128], rhs [128,7,2,512]).
    ACT : exact exp of R0 (1 instr) + head of R1 ([1536,AW2)), both
          with accum_out row sums.
    DVE : Schraudolph exp of [AW2,N) (tensor_scalar (x*S+B) -> int16
          bitcast bf16, ~2% rel err, tolerance is 2e-2); fused
          colaccV+rowsum via scalar_tensor_tensor accum (cumulative
          rowsums S_t, per-tile rowsums = S_t - S_{t-1}); bf16
          tensor_tensor colacc of [0,CVA); diag chunk (253 cols).
    Pool: bf16 tensor_tensor colacc of [CVA,AW2) (GPSIMD: no PSUM, no
          stt, no tensor_scalar -> plain adds only).

Finalize: rowsums = rs0+rs1+(S_t - S_{t-1}) -> Ln+accum; colacc column
sums via ones-matmuls into 4 partition-rows (0/32/64/96) of a [128,885]
PSUM tile pre-memset to 1.0 (Ln(1)=0 elsewhere), single Ln+accum;
partition-reduce by one f32 ones-matmul; DMA [1,3] out.
"""

import numpy as np
import ml_dtypes

B = 8
C = 256
N = 3540
TEMP = 0.2
MT = 128
NT = (N + MT - 1) // MT          # 28 m-tiles (last has 84 rows)
NB = 7                            # rhs 512-col blocks (last holds 468)
MP_PAD = NT * MT                  # 3584, lhsT m padded

R0 = 1536                         # region 0 = [0, R0), region 1 = [R0, N)
AW2 = 2612                        # ACT covers [0, AW2); DVE schrau [AW2, N)
CVA = 902                         # DVE colacc [0, CVA); Pool [CVA, AW2)

# Schraudolph: i16 = trunc(raw_dot * SCH_S + SCH_B); bitcast bf16
# approximates exp(TEMP * raw_dot).  C = -6.5 calibrated for minimal bias.
SCH_S = TEMP * 128.0 / float(np.log(2.0))
SCH_B = 16256.0 - 6.5

DIAG_W = (2 * N + NT - 1) // NT   # 253-wide diag chunk per m-tile
# fin columns: 0 diag, 1 rowlog, 2 collog
FIN_W = 3

_prog_cache = {}

# (block j, in-block offset, width) chunk lists per region; every PSUM
# output chunk stays inside one 2KB bank and every rhs chunk inside one
# 512-col block.
_R0_CHUNKS = [(0, 0, 512), (1, 0, 512), (2, 0, 512)]
_R1_CHUNKS = [(3, 0, 512), (4, 0, 512), (5, 0, 512), (6, 0, 468)]


def _build_program():
    import concourse.bacc as bacc
    import concourse.tile as tile
    from concourse import mybir

    dt = mybir.dt
    f32 = dt.float32
    bf16 = dt.bfloat16
    i16 = dt.int16
    fp8 = dt.float8e4
    Exp = mybir.ActivationFunctionType.Exp
    Ln = mybir.ActivationFunctionType.Ln
    MULT = mybir.AluOpType.mult
    ADD = mybir.AluOpType.add
    SUB = mybir.AluOpType.subtract
    DR = mybir.MatmulPerfMode.DoubleRow

    nc = bacc.Bacc(
        "TRN2", target_bir_lowering=False, debug=False, num_devices=B)
    d1 = nc.dram_tensor("d1", [MT, NT, 2, MT], fp8, kind="ExternalInput")
    d2 = nc.dram_tensor("d2", [MT, NB, 2, 512], fp8, kind="ExternalInput")
    b1 = nc.dram_tensor("b1", [MT, 2 * N], bf16, kind="ExternalInput")
    b2 = nc.dram_tensor("b2", [MT, 2 * N], bf16, kind="ExternalInput")
    out = nc.dram_tensor("out", [1, FIN_W], f32, kind="ExternalOutput")

    with tile.TileContext(nc) as tc:
        with (
            tc.tile_pool(name="persist", bufs=1) as persist,
            tc.tile_pool(name="etiles", bufs=2) as etiles,
            tc.tile_pool(name="small", bufs=1) as small,
        ):
            d1_sb = persist.tile([MT, NT, 2, MT], fp8, name="d1_sb")
            d2_sb = persist.tile([MT, NB, 2, 512], fp8, name="d2_sb")
            b1_sb = persist.tile([MT, 2 * N], bf16, name="b1_sb")
            b2_sb = persist.tile([MT, 2 * N], bf16, name="b2_sb")

            # fp8 operands first (tile 0 needs all of d2 + head of d1).
            nc.sync.dma_start(out=d2_sb, in_=d2[:, :, :, :])
            nc.scalar.dma_start(out=d1_sb[:, 0:4, :, :], in_=d1[:, 0:4, :, :])
            nc.scalar.dma_start(out=d1_sb[:, 4:NT, :, :],
                                in_=d1[:, 4:NT, :, :])
            # bf16 copies (diag only, 253 cols per tile).
            nc.sync.dma_start(out=b1_sb, in_=b1[:, :])
            nc.scalar.dma_start(out=b2_sb, in_=b2[:, :])

            colacc = persist.tile([MT, N], bf16, name="colacc")
            nc.vector.memset(colacc, 0.0)

            # 0.5-init: rows of the last (84-row) m-tile that do not exist
            # end up with rowsum 0.5+0.5+0 = 1.0 -> Ln contributes 0.
            rs0 = persist.tile([MT, NT], f32, name="rs0")
            nc.vector.memset(rs0, 0.5)
            rs1 = persist.tile([MT, NT], f32, name="rs1")
            nc.vector.memset(rs1, 0.5)
            # cumulative rowsums of colaccV; per-tile rowsum = S_t - S_{t-1}
            S = persist.tile([MT, NT], f32, name="S")
            nc.vector.memset(S, 0.0)
            rs_last = persist.tile([MT, 1], f32, name="rs_last")
            nc.vector.memset(rs_last, 0.0)

            fin = small.tile([MT, FIN_W], f32, name="fin")
            nc.vector.memset(fin, 0.0)
            diagparts = small.tile([MT, NT], f32, name="diagparts")
            nc.vector.memset(diagparts, 0.0)

            ones_bf = small.tile([MT, 1], bf16, name="ones_bf")
            nc.vector.memset(ones_bf, 1.0)
            ones_f32 = small.tile([MT, 1], f32, name="ones_f32")
            nc.vector.memset(ones_f32, 1.0)

            scratchD = small.tile([MT, DIAG_W], bf16, name="scratchD")

            with (
                tc.tile_pool(name="ps0", bufs=1, space="PSUM") as psp0,
                tc.tile_pool(name="ps1", bufs=1, space="PSUM") as psp1,
            ):
                for t in range(NT):
                    mp = min(MT, N - t * MT)
                    p0 = psp0.tile([MT, R0], f32, tag="p0", name="p0")
                    p1t = psp1.tile([MT, N - R0], f32, tag="p1", name="p1")
                    lhsT = d1_sb[:, t, :, 0:mp]
                    for (j, o, w) in _R0_CHUNKS:
                        g = 512 * j + o
                        nc.tensor.matmul(
                            p0[:mp, g:g + w],
                            lhsT=lhsT, rhs=d2_sb[:, j, :, o:o + w],
                            start=True, stop=True, perf_mode=DR)
                    for (j, o, w) in _R1_CHUNKS:
                        g = 512 * j + o - R0
                        nc.tensor.matmul(
                            p1t[:mp, g:g + w],
                            lhsT=lhsT, rhs=d2_sb[:, j, :, o:o + w],
                            start=True, stop=True, perf_mode=DR)

                    e = etiles.tile([MT, N], bf16, tag="e", name="e")
                    # ACT: exact exp + rowsums, R0 then R1 head
                    nc.scalar.activation(
                        out=e[:mp, 0:R0], in_=p0[:mp, :], func=Exp,
                        scale=TEMP, accum_out=rs0[:mp, t:t + 1])
                    nc.scalar.activation(
                        out=e[:mp, R0:AW2], in_=p1t[:mp, 0:AW2 - R0],
                        func=Exp, scale=TEMP,
                        accum_out=rs1[:mp, t:t + 1])
                    # DVE: Schraudolph exp for [AW2, N)
                    nc.vector.tensor_scalar(
                        out=e[:mp, AW2:N].bitcast(i16),
                        in0=p1t[:mp, AW2 - R0:N - R0],
                        scalar1=SCH_S, scalar2=SCH_B, op0=MULT, op1=ADD)
                    if t < NT - 1:
                        # DVE: fused colaccV += exp, cumulative rowsum S_t
                        nc.vector.scalar_tensor_tensor(
                            out=colacc[:mp, AW2:N], in0=e[:mp, AW2:N],
                            scalar=1.0, in1=colacc[:mp, AW2:N],
                            op0=MULT, op1=ADD, accum_out=S[:mp, t:t + 1])
                    else:
                        # last (84-row) tile: separate rowsum + plain add so
                        # the S-difference trick never touches partitions>=84
                        nc.vector.tensor_scalar(
                            out=e[:mp, AW2:N], in0=e[:mp, AW2:N],
                            scalar1=1.0, scalar2=None, op0=MULT, op1=ADD,
                            accum_out=rs_last[:mp, 0:1])
                        nc.vector.tensor_add(
                            colacc[:mp, AW2:N], colacc[:mp, AW2:N],
                            e[:mp, AW2:N])
                    # DVE: colacc [0, CVA)
                    nc.vector.tensor_add(
                        colacc[:mp, 0:CVA], colacc[:mp, 0:CVA],
                        e[:mp, 0:CVA])
                    # Pool: colacc [CVA, AW2)
                    nc.gpsimd.tensor_add(
                        colacc[:mp, CVA:AW2], colacc[:mp, CVA:AW2],
                        e[:mp, CVA:AW2])
                    # DVE: one diag chunk per tile
                    s = t * DIAG_W
                    w = min(DIAG_W, 2 * N - s)
                    nc.vector.scalar_tensor_tensor(
                        out=scratchD[:, 0:w], in0=b1_sb[:, s:s + w],
                        scalar=1.0, in1=b2_sb[:, s:s + w],
                        op0=MULT, op1=MULT,
                        accum_out=diagparts[:, t:t + 1])

            # ---- finalize ----
            nc.vector.tensor_scalar(
                out=diagparts, in0=diagparts, scalar1=1.0, scalar2=None,
                op0=MULT, op1=ADD, accum_out=fin[:, 0:1])
            rowsums = small.tile([MT, NT], f32, name="rowsums")
            nc.vector.tensor_add(rowsums, rs0, rs1)
            nc.vector.tensor_add(rowsums, rowsums, S)
            nc.vector.tensor_tensor(
                out=rowsums[:, 1:NT - 1], in0=rowsums[:, 1:NT - 1],
                in1=S[:, 0:NT - 2], op=SUB)
            nc.vector.tensor_add(
                rowsums[:, NT - 1:NT], rowsums[:, NT - 1:NT], rs_last)
            rl = small.tile([MT, NT], f32, name="rl")
            nc.scalar.activation(out=rl, in_=rowsums, func=Ln,
                                 accum_out=fin[:, 1:2])

            with tc.tile_pool(name="psF", bufs=1, space="PSUM") as psF:
                # colacc column sums: 4 blocks of 885 cols, one per
                # partition-row 0/32/64/96; unused partitions memset to
                # 1.0 so a single full-width Ln contributes 0 there.
                BW = N // 4   # 885
                psF2 = psF.tile([MT, BW], f32, tag="psF2", name="psF2")
                nc.vector.memset(psF2, 1.0)
                for j in range(4):
                    base = j * BW
                    for (off, w) in ((0, 512), (512, BW - 512)):
                        nc.tensor.matmul(
                            psF2[32 * j:32 * j + 1, off:off + w],
                            lhsT=ones_bf,
                            rhs=colacc[:, base + off:base + off + w],
                            start=True, stop=True,
                            tile_position=(0, 32 * j))
                clF = small.tile([MT, BW], f32, name="clF")
                nc.scalar.activation(out=clF, in_=psF2, func=Ln,
                                     accum_out=fin[:, 2:3])

                dr_ps = psF.tile([1, FIN_W], f32, tag="drps", name="drps")
                nc.tensor.matmul(dr_ps[0:1, :], lhsT=ones_f32,
                                 rhs=fin, start=True, stop=True)
                outsb = small.tile([1, FIN_W], f32, name="outsb")
                nc.vector.tensor_copy(outsb, dr_ps[0:1, :])
                nc.sync.dma_start(out=out[:, :], in_=outsb)

    nc.compile()
    return nc


def _get_program():
    if "nc" not in _prog_cache:
        _prog_cache["nc"] = _build_program()
    return _prog_cache["nc"]


def _prep_in_maps(inputs):
    p1 = np.asarray(inputs["p1"], dtype=np.float32)
    p2 = np.asarray(inputs["p2"], dtype=np.float32)
    y1 = np.asarray(inputs["y1"]).astype(np.int64)
    x1 = np.asarray(inputs["x1"]).astype(np.int64)
    y2 = np.asarray(inputs["y2"]).astype(np.int64)
    x2 = np.asarray(inputs["x2"]).astype(np.int64)

    des1 = p1[:, :, y1, x1]                      # [B, C, N] f32
    des2 = p2[:, :, y2, x2]
    # DoubleRow pair layout [p, i] with small strides:
    # lhsT blocks: [128, NT, 2, 128]; rhs blocks: [128, NB, 2, 512]
    padm = np.zeros((B, C, MP_PAD - N), np.float32)
    dd1 = np.concatenate([des1, padm], axis=2).reshape(B, 2, MT, NT, MT)
    l1 = dd1.transpose(0, 2, 3, 1, 4)            # [B, 128, NT, 2, 128]
    padn = np.zeros((B, C, NB * 512 - N), np.float32)
    dd2 = np.concatenate([des2, padn], axis=2).reshape(B, 2, MT, NB, 512)
    r2 = dd2.transpose(0, 2, 3, 1, 4)            # [B, 128, NB, 2, 512]
    f8_1 = np.ascontiguousarray(l1).astype(ml_dtypes.float8_e4m3fn)
    f8_2 = np.ascontiguousarray(r2).astype(ml_dtypes.float8_e4m3fn)
    # bf16 copies for diag: [128, 2, N] flattened (pair-major free dims)
    dr1 = des1.reshape(B, 2, MT, N).transpose(0, 2, 1, 3)
    dr2 = des2.reshape(B, 2, MT, N).transpose(0, 2, 1, 3)
    bf_1 = np.ascontiguousarray(dr1.reshape(B, MT, 2 * N)).astype(
        ml_dtypes.bfloat16)
    bf_2 = np.ascontiguousarray(dr2.reshape(B, MT, 2 * N)).astype(
        ml_dtypes.bfloat16)
    return [
        {"d1": f8_1[b], "d2": f8_2[b], "b1": bf_1[b], "b2": bf_2[b]}
        for b in range(B)
    ]


def _combine(results):
    total = 0.0
    for b in range(B):
        v = np.asarray(results[b]["out"], dtype=np.float64).ravel()
        total += 2.0 * TEMP * v[0] - v[1] - v[2]
    return np.float32(-total / (B * N))


def kernel(**inputs) -> np.ndarray:
    from concourse.bass_utils import run_bass_kernel_spmd

    nc = _get_program()
    in_maps = _prep_in_maps(inputs)
    res = run_bass_kernel_spmd(nc, in_maps, list(range(B)))
    return _combine(res.results)


# revision 12
# speedup vs baseline: 1.6272x; 1.4094x over previous
"""Trainium2 Bass kernel for the DescriptorLoss dual-softmax loss.

Math (per batch element b):
    des1 = p1[b][:, y1, x1]            # [C=256, N=3540]
    des2 = p2[b][:, y2, x2]            # [C, N]
    dist = TEMP * des1.T @ des2        # [N, N]
    loss_b = 2*trace(dist) - sum_m lse_row[m] - sum_n lse_col[n]
    loss   = -(sum_b loss_b) / (B*N)

Sharding: data-parallel over the batch dim, one batch element per
NeuronCore (B == 8 == n_cores).  Host gathers descriptors, quantizes to
fp8e4m3 in DoubleRow block layouts, runs the SPMD program, and combines
the per-core partial sums (loss tolerance 2e-2; fp8 dist + a small
Schraudolph-exp slice keep rel err ~5e-4, validated vs reference).

Per-core structure (two PSUM regions R0=[0,1536), R1=[1536,3540),
pipelined as in the proven baseline; per-tile steady state ~3.7us):
    PE : dist via fp8 DoubleRow matmuls - contraction 256 = 128
         partitions x 2 interleaved rows at 2x rate.  Block layouts
         keep the pair stride small (lhsT [128,28,2,128] blocks per
         m-tile, rhs [128,7,2,512] blocks per 512 columns), which is
         required for full-rate DR (large pair strides run at 1x and
         are rejected >16B-misaligned).  Ones-matmul finalize.
    ACT: exact exp of R0 and of [1536, AW2), accum_out row sums.
    DVE: Schraudolph exp of the tail [AW2, N) (tensor_scalar
         (x*S + B) -> int16 bitcast bf16), its rowsum pass (accum),
         both colacc += exp adds (bf16 2x), and the exact-diag stt.
"""

import numpy as np
import ml_dtypes

B = 8
C = 256
N = 3540
TEMP = 0.2
MT = 128
NT = (N + MT - 1) // MT          # 28 m-tiles (last has 84 rows)
NB = 7                            # rhs 512-col blocks (last holds 468)
MP_PAD = NT * MT                  # 3584, lhsT m padded

R0 = 1536                         # region 0 = [0, R0), region 1 = [R0, N)
AW2 = 3090                        # ACT exp [0, AW2); DVE Schraudolph [AW2, N)

# Schraudolph: i16 = trunc(raw_dot * SCH_S + SCH_B); bitcast bf16
# approximates exp(TEMP * raw_dot).  C = -6.5 calibrated for minimal bias.
SCH_S = TEMP * 128.0 / float(np.log(2.0))
SCH_B = 16256.0 - 6.5

_prog_cache = {}

# (block j, in-block offset, width) chunk lists per region: every PSUM
# output chunk stays inside one 2KB bank, every rhs chunk inside one
# 512-col block.
_R0_CHUNKS = [(0, 0, 512), (1, 0, 512), (2, 0, 512)]
_R1_CHUNKS = [(3, 0, 512), (4, 0, 512), (5, 0, 512), (6, 0, 468)]


def _mm_chunks(width):
    out = []
    off = 0
    while off < width:
        w = min(512, width - off)
        out.append((off, w))
        off += w
    return out


def _build_program():
    import concourse.bacc as bacc
    import concourse.tile as tile
    from concourse import mybir

    dt = mybir.dt
    f32 = dt.float32
    bf16 = dt.bfloat16
    i16 = dt.int16
    fp8 = dt.float8e4
    Exp = mybir.ActivationFunctionType.Exp
    Ln = mybir.ActivationFunctionType.Ln
    MULT = mybir.AluOpType.mult
    ADD = mybir.AluOpType.add
    DR = mybir.MatmulPerfMode.DoubleRow

    nc = bacc.Bacc(
        "TRN2", target_bir_lowering=False, debug=False, num_devices=B)
    d1 = nc.dram_tensor("d1", [MT, NT, 2, MT], fp8, kind="ExternalInput")
    d2 = nc.dram_tensor("d2", [MT, NB, 2, 512], fp8, kind="ExternalInput")
    b1 = nc.dram_tensor("b1", [MT, 2 * N], bf16, kind="ExternalInput")
    b2 = nc.dram_tensor("b2", [MT, 2 * N], bf16, kind="ExternalInput")
    out = nc.dram_tensor("out", [1, 3], f32, kind="ExternalOutput")

    with tile.TileContext(nc) as tc:
        with (
            tc.tile_pool(name="persist", bufs=1) as persist,
            tc.tile_pool(name="etiles", bufs=2) as etiles,
            tc.tile_pool(name="small", bufs=1) as small,
        ):
            d1_sb = persist.tile([MT, NT, 2, MT], fp8, name="d1_sb")
            d2_sb = persist.tile([MT, NB, 2, 512], fp8, name="d2_sb")
            b1_sb = persist.tile([MT, 2 * N], bf16, name="b1_sb")
            b2_sb = persist.tile([MT, 2 * N], bf16, name="b2_sb")

            # fp8 operands first (tile 0 needs all of d2 + head of d1).
            nc.sync.dma_start(out=d2_sb, in_=d2[:, :, :, :])
            nc.scalar.dma_start(out=d1_sb[:, 0:4, :, :], in_=d1[:, 0:4, :, :])
            nc.scalar.dma_start(out=d1_sb[:, 4:NT, :, :],
                                in_=d1[:, 4:NT, :, :])
            # bf16 copies for the diag term.
            nc.sync.dma_start(out=b1_sb, in_=b1[:, :])
            nc.scalar.dma_start(out=b2_sb, in_=b2[:, :])

            colacc = persist.tile([MT, N], bf16, name="colacc")
            nc.vector.memset(colacc, 0.0)

            # rsparts blocks: 0 = ACT R0, 1 = ACT R1-head, 2 = DVE tail.
            # 0.5/0.5/0.0-init: missing rows of the last m-tile sum to 1.0
            # -> Ln contributes 0.
            rsparts = small.tile([MT, 3 * NT], f32, name="rsparts")
            nc.vector.memset(rsparts[:, 0:2 * NT], 0.5)
            nc.vector.memset(rsparts[:, 2 * NT:3 * NT], 0.0)

            ones_bf = small.tile([MT, 1], bf16, name="ones_bf")
            nc.vector.memset(ones_bf, 1.0)
            ones_f32 = small.tile([MT, 1], f32, name="ones_f32")
            nc.vector.memset(ones_f32, 1.0)

            # fin[:,0] = diag partial, fin[:,1] = sum of row-logs partial
            fin = small.tile([MT, 2], f32, name="fin")

            # diag = sum over C of des1*des2 per column: one stt per half,
            # off the critical path (overlaps DMA + PE ramp).
            scratch = persist.tile([MT, N], bf16, name="scratch")
            diag0 = small.tile([MT, 1], f32, name="diag0")
            diag1 = small.tile([MT, 1], f32, name="diag1")
            nc.vector.scalar_tensor_tensor(
                out=scratch, in0=b1_sb[:, 0:N], scalar=1.0,
                in1=b2_sb[:, 0:N], op0=MULT, op1=MULT, accum_out=diag0)
            nc.vector.scalar_tensor_tensor(
                out=scratch, in0=b1_sb[:, N:2 * N], scalar=1.0,
                in1=b2_sb[:, N:2 * N], op0=MULT, op1=MULT, accum_out=diag1)
            nc.vector.tensor_add(fin[:, 0:1], diag0, diag1)

            with (
                tc.tile_pool(name="ps0", bufs=1, space="PSUM") as psp0,
                tc.tile_pool(name="ps1", bufs=1, space="PSUM") as psp1,
            ):
                for t in range(NT):
                    mp = min(MT, N - t * MT)
                    p0 = psp0.tile([MT, R0], f32, tag="p0", name="p0")
                    p1t = psp1.tile([MT, N - R0], f32, tag="p1", name="p1")
                    lhsT = d1_sb[:, t, :, 0:mp]
                    for (j, o, w) in _R0_CHUNKS:
                        g = 512 * j + o
                        nc.tensor.matmul(
                            p0[:mp, g:g + w],
                            lhsT=lhsT, rhs=d2_sb[:, j, :, o:o + w],
                            start=True, stop=True, perf_mode=DR)
                    for (j, o, w) in _R1_CHUNKS:
                        g = 512 * j + o - R0
                        nc.tensor.matmul(
                            p1t[:mp, g:g + w],
                            lhsT=lhsT, rhs=d2_sb[:, j, :, o:o + w],
                            start=True, stop=True, perf_mode=DR)

                    e = etiles.tile([MT, N], bf16, tag="e", name="e")
                    # ACT: exact exp + rowsums, R0 then R1 head
                    nc.scalar.activation(
                        out=e[:mp, 0:R0], in_=p0[:mp, :], func=Exp,
                        scale=TEMP, accum_out=rsparts[:mp, t:t + 1])
                    nc.scalar.activation(
                        out=e[:mp, R0:AW2], in_=p1t[:mp, 0:AW2 - R0],
                        func=Exp, scale=TEMP,
                        accum_out=rsparts[:mp, NT + t:NT + t + 1])
                    # DVE: Schraudolph exp tail + its rowsum pass
                    nc.vector.tensor_scalar(
                        out=e[:mp, AW2:N].bitcast(i16),
                        in0=p1t[:mp, AW2 - R0:N - R0],
                        scalar1=SCH_S, scalar2=SCH_B, op0=MULT, op1=ADD)
                    nc.vector.tensor_scalar(
                        out=e[:mp, AW2:N], in0=e[:mp, AW2:N],
                        scalar1=1.0, scalar2=None, op0=MULT, op1=ADD,
                        accum_out=rsparts[:mp, 2 * NT + t:2 * NT + t + 1])
                    # DVE: colacc adds per region
                    nc.vector.tensor_add(
                        colacc[:mp, 0:R0], colacc[:mp, 0:R0], e[:mp, 0:R0])
                    nc.vector.tensor_add(
                        colacc[:mp, R0:N], colacc[:mp, R0:N], e[:mp, R0:N])

            # ---- finalize ----
            rowsums = small.tile([MT, NT], f32, name="rowsums")
            nc.vector.tensor_add(
                rowsums, rsparts[:, 0:NT], rsparts[:, NT:2 * NT])
            nc.vector.tensor_add(
                rowsums, rowsums, rsparts[:, 2 * NT:3 * NT])
            rl = small.tile([MT, NT], f32, name="rl")
            nc.scalar.activation(out=rl, in_=rowsums, func=Ln,
                                 accum_out=fin[:, 1:2])

            with tc.tile_pool(name="psF", bufs=1, space="PSUM") as psF:
                # column sums: ones-matmuls into one 7-bank PSUM strip,
                # then a single Ln whose accum_out is sum(log(colsum)).
                csum = psF.tile([1, 3584], f32, tag="csum", name="csum")
                for (off, w) in _mm_chunks(N):
                    nc.tensor.matmul(csum[0:1, off:off + w], lhsT=ones_bf,
                                     rhs=colacc[:, off:off + w],
                                     start=True, stop=True)
                cl = small.tile([1, N], f32, name="cl")
                clsum = small.tile([1, 1], f32, name="clsum")
                nc.scalar.activation(out=cl, in_=csum[0:1, 0:N], func=Ln,
                                     accum_out=clsum)

                # partition-reduce diag and row-log partials in one matmul
                dr_ps = psF.tile([1, 2], f32, tag="drps", name="dr_ps")
                nc.tensor.matmul(dr_ps[0:1, 0:2], lhsT=ones_f32,
                                 rhs=fin[:, 0:2], start=True, stop=True)

                outsb = small.tile([1, 3], f32, name="outsb")
                nc.vector.tensor_copy(outsb[0:1, 0:2], dr_ps[0:1, 0:2])
                nc.vector.tensor_copy(outsb[0:1, 2:3], clsum)
                nc.sync.dma_start(out=out[:, :], in_=outsb)

    nc.compile()
    return nc


def _get_program():
    if "nc" not in _prog_cache:
        _prog_cache["nc"] = _build_program()
    return _prog_cache["nc"]


def _prep_in_maps(inputs):
    p1 = np.asarray(inputs["p1"], dtype=np.float32)
    p2 = np.asarray(inputs["p2"], dtype=np.float32)
    y1 = np.asarray(inputs["y1"]).astype(np.int64)
    x1 = np.asarray(inputs["x1"]).astype(np.int64)
    y2 = np.asarray(inputs["y2"]).astype(np.int64)
    x2 = np.asarray(inputs["x2"]).astype(np.int64)

    des1 = p1[:, :, y1, x1]                      # [B, C, N] f32
    des2 = p2[:, :, y2, x2]
    # DoubleRow pair layouts with small strides:
    # lhsT blocks [128, NT, 2, 128]; rhs blocks [128, NB, 2, 512]
    padm = np.zeros((B, C, MP_PAD - N), np.float32)
    dd1 = np.concatenate([des1, padm], axis=2).reshape(B, 2, MT, NT, MT)
    l1 = dd1.transpose(0, 2, 3, 1, 4)
    padn = np.zeros((B, C, NB * 512 - N), np.float32)
    dd2 = np.concatenate([des2, padn], axis=2).reshape(B, 2, MT, NB, 512)
    r2 = dd2.transpose(0, 2, 3, 1, 4)
    f8_1 = np.ascontiguousarray(l1).astype(ml_dtypes.float8_e4m3fn)
    f8_2 = np.ascontiguousarray(r2).astype(ml_dtypes.float8_e4m3fn)
    # bf16 copies for diag: [128, 2, N] flattened
    dr1 = des1.reshape(B, 2, MT, N).transpose(0, 2, 1, 3)
    dr2 = des2.reshape(B, 2, MT, N).transpose(0, 2, 1, 3)
    bf_1 = np.ascontiguousarray(dr1.reshape(B, MT, 2 * N)).astype(
        ml_dtypes.bfloat16)
    bf_2 = np.ascontiguousarray(dr2.reshape(B, MT, 2 * N)).astype(
        ml_dtypes.bfloat16)
    return [
        {"d1": f8_1[b], "d2": f8_2[b], "b1": bf_1[b], "b2": bf_2[b]}
        for b in range(B)
    ]


def _combine(results):
    total = 0.0
    for b in range(B):
        d, r, c = (float(v) for v in
                   np.asarray(results[b]["out"], dtype=np.float64).ravel())
        total += 2.0 * TEMP * d - r - c
    return np.float32(-total / (B * N))


def kernel(**inputs) -> np.ndarray:
    from concourse.bass_utils import run_bass_kernel_spmd

    nc = _get_program()
    in_maps = _prep_in_maps(inputs)
    res = run_bass_kernel_spmd(nc, in_maps, list(range(B)))
    return _combine(res.results)
